# revision 14
# baseline (speedup 1.0000x reference)
# Trainium2 Bass kernel for nn_MergeEncoder_47768626266491 (GIN message passing).
#
# Math notes that shape the implementation:
#   * The edge list is dense: src = repeat(arange(N), N-1), dst = tile(arange(1,N), N).
#     Every destination node d >= 1 receives ALL N source nodes, node 0 receives none.
#     So segment_sum(h[src], dst) == broadcast(colsum(h)) for rows 1..N-1, 0 for row 0.
#   * By linearity, (x + 1_{i>0} S) @ W = x@W + 1_{i>0} (S@W): the aggregation is folded
#     into a per-output-channel bias correction c = W.T @ S applied to columns >= 1.
#   * colsum(bn1_out) == N * be1 exactly (BN output sums to zero per channel), so the
#     second aggregation's S2 = N*be1 without a reduction over bn1.
#
# Distribution: the problem is tiny (1.3 MB of I/O, ~0.4 GFLOP total); the 8-core
# all-reduce latency floor (~10 us per collective, 3 would be needed) far exceeds the
# whole computation. Each core therefore runs the full network redundantly (SPMD,
# no collectives) and the host takes core 0's output.
#
# Layout: channels on partitions, nodes on the free axis (x is transposed on the PE).
# Matmuls run in float32r (1 cycle/row on the PE vs 4 for fp32); f32r is a rounded
# 4-byte format, so every tile feeding an f32r matmul is *written* as f32r by its
# producer (walrus verifies this).

import numpy as np

N, F, H = 1024, 128, 256
NCORES = 8
BN_EPS = 1e-5
P = 128  # partitions


def _build_bass(debug=False):
    from contextlib import ExitStack

    import concourse.bacc as bacc
    import concourse.tile as tile
    from concourse import mybir
    from concourse.masks import make_identity
    import concourse.hw_specs as hw_specs

    fp32 = mybir.dt.float32
    f32r = mybir.dt.float32r
    RELU = mybir.ActivationFunctionType.Relu
    SQRT = mybir.ActivationFunctionType.Sqrt
    ADD = mybir.AluOpType.add
    MULT = mybir.AluOpType.mult
    MAX = mybir.AluOpType.max
    X = mybir.AxisListType.X

    # All ACT functions used here (Relu/Sqrt/Copy) live in the sqrt_and_others
    # set; the default chooser assigns Relu to set 0 which costs a second 1.3us
    # ACT_TABLE_LOAD mid-kernel. Blank every other set (list positions, and
    # hence act_func_set_ids, are preserved) so one load covers everything.
    _orig_tables = hw_specs.get_activation_tables("gen3")

    def _only_sqrt_tables(arch):
        return {name: (fns if name == "sqrt_and_others" else set())
                for name, fns in _orig_tables.items()}

    bacc.get_activation_tables = _only_sqrt_tables

    nc = bacc.Bacc()

    x_h = nc.declare_dram_parameter("x", [N, F], fp32, isOutput=False)
    W1a_h = nc.declare_dram_parameter("W1a", [F, H], fp32, isOutput=False)
    b1a_h = nc.declare_dram_parameter("b1a", [H], fp32, isOutput=False)
    W1b_h = nc.declare_dram_parameter("W1b", [H, H], fp32, isOutput=False)
    b1b_h = nc.declare_dram_parameter("b1b", [H], fp32, isOutput=False)
    g1_h = nc.declare_dram_parameter("g1", [H], fp32, isOutput=False)
    be1_h = nc.declare_dram_parameter("be1", [H], fp32, isOutput=False)
    W2a_h = nc.declare_dram_parameter("W2a", [H, H], fp32, isOutput=False)
    b2a_h = nc.declare_dram_parameter("b2a", [H], fp32, isOutput=False)
    W2b_h = nc.declare_dram_parameter("W2b", [H, F], fp32, isOutput=False)
    b2b_h = nc.declare_dram_parameter("b2b", [F], fp32, isOutput=False)
    g2_h = nc.declare_dram_parameter("g2", [F], fp32, isOutput=False)
    be2_h = nc.declare_dram_parameter("be2", [F], fp32, isOutput=False)
    out_h = nc.declare_dram_parameter("out", [F], fp32, isOutput=True)

    dbg = {}
    if debug:
        dbg["S1"] = nc.declare_dram_parameter("dbg_S1", [P], fp32, isOutput=True)
        dbg["mv1"] = nc.declare_dram_parameter("dbg_mv1", [P, 4], fp32, isOutput=True)
        dbg["bn1T"] = nc.declare_dram_parameter("dbg_bn1T", [P, 2, N], fp32, isOutput=True)
        dbg["r2T"] = nc.declare_dram_parameter("dbg_r2T", [P, N], fp32, isOutput=True)

    NT = N // P  # 8 node tiles
    CH = 512  # node-chunk width for matmuls

    with tile.TileContext(nc) as tc, ExitStack() as ctx:
        main = ctx.enter_context(tc.tile_pool(name="main", bufs=1))
        ps_tr = ctx.enter_context(tc.tile_pool(name="ps_tr", bufs=2, space="PSUM"))
        ps_mm = ctx.enter_context(tc.tile_pool(name="ps_mm", bufs=3, space="PSUM"))

        # ---- loads ------------------------------------------------------
        ident = main.tile([P, P], fp32, tag="ident")
        make_identity(nc, ident)

        # x: 8 node-tile DMAs alternating across the two HWDGE rings so the
        # first transpose can start as soon as tile 0 lands.
        xt = main.tile([P, NT, P], fp32, tag="xt")
        for t in range(NT):
            eng = nc.sync if t % 2 == 0 else nc.scalar
            eng.dma_start(out=xt[:, t, :], in_=x_h[t * P:(t + 1) * P, :])

        W1a_st = main.tile([P, H], fp32, tag="W1a_st")
        nc.sync.dma_start(out=W1a_st, in_=W1a_h[:, :])
        W1b_st = main.tile([P, 2, H], fp32, tag="W1b_st")
        nc.sync.dma_start(out=W1b_st, in_=W1b_h[:, :].rearrange("(k p) h -> p k h", p=P))
        W2a_st = main.tile([P, 2, H], fp32, tag="W2a_st")
        nc.sync.dma_start(out=W2a_st, in_=W2a_h[:, :].rearrange("(k p) h -> p k h", p=P))
        W2b_st = main.tile([P, 2, F], fp32, tag="W2b_st")
        nc.sync.dma_start(out=W2b_st, in_=W2b_h[:, :].rearrange("(k p) h -> p k h", p=P))

        # bias/scale vectors: one single-row DMA each on the gpsimd SWDGE ring,
        # transposed to per-partition columns on the PE later.
        vrows = main.tile([8, H], fp32, tag="vrows")
        vhandles = [b1a_h, b1b_h, g1_h, be1_h, b2a_h, b2b_h, g2_h, be2_h]
        for k, h_ in enumerate(vhandles):
            L = h_.shape[0]
            nc.gpsimd.dma_start(out=vrows[k:k + 1, 0:L],
                                in_=h_[:].rearrange("(o f) -> o f", o=1))

        eps_sb = main.tile([P, 1], fp32, tag="eps")
        nc.vector.memset(eps_sb, BN_EPS)
        # dummy sqrt: forces the single ACT table load to happen up front,
        # overlapped with the input DMAs
        warm_sb = main.tile([P, 1], fp32, tag="warm")
        nc.scalar.activation(warm_sb, eps_sb, SQRT)

        # f32r copies of the weights (DVE cast, 2x mode)
        W1a_sb = main.tile([P, H], f32r, tag="W1a")
        nc.vector.tensor_copy(out=W1a_sb, in_=W1a_st)
        W1b_sb = main.tile([P, 2, H], f32r, tag="W1b")
        nc.vector.tensor_copy(out=W1b_sb, in_=W1b_st)
        W2a_sb = main.tile([P, 2, H], f32r, tag="W2a")
        nc.gpsimd.tensor_copy(out=W2a_sb, in_=W2a_st)
        W2b_sb = main.tile([P, 2, F], f32r, tag="W2b")
        nc.gpsimd.tensor_copy(out=W2b_sb, in_=W2b_st)

        # ---- transpose x -> xT [128 ch, 1024 nodes] (f32r) --------------
        # evacuations split: even tiles on DVE, odd tiles on ACT
        xT = main.tile([P, N], f32r, tag="xT")
        S1p = main.tile([P, NT], fp32, tag="S1p")
        S1 = main.tile([P, 1], fp32, tag="S1")
        CPY = mybir.ActivationFunctionType.Copy
        _last_tr = None
        for t in range(NT):
            pst = ps_tr.tile([P, P], fp32, tag="pst")
            _last_tr = nc.tensor.transpose(pst, xt[:, t, :], ident)
            # evac on ACT with a free running row-sum -> S1 partials
            nc.scalar.activation(xT[:, t * P:(t + 1) * P].bitcast(f32r), pst, CPY,
                                 accum_out=S1p[:, t:t + 1])
        nc.vector.reduce_sum(out=S1, in_=S1p, axis=X)

        # ---- MLP1 layer A ------------------------------------------------
        from concourse.tile_rust import add_dep_helper
        ps1a = []
        for m in range(2):
            ps = ps_mm.tile([P, N], fp32, tag="mm")
            for c in range(2):
                mm = nc.tensor.matmul(
                    ps[:, c * CH:(c + 1) * CH],
                    W1a_sb[:, m * P:(m + 1) * P],
                    xT[:, c * CH:(c + 1) * CH],
                    start=True, stop=True,
                )
                if m == 0 and c == 0 and _last_tr is not None:
                    # keep all transposes ahead of the MLP in the PE stream so
                    # S1 (and with it the c1 bias path) resolves early
                    add_dep_helper(_last_tr.ins, mm.ins, sync=False,
                                   reason="transposes before MLP1A")
            ps1a.append(ps)

        # vector transposes (after MM1A in the PE stream; only needed for the
        # relu biases)
        vcols = main.tile([P, 2, 8], fp32, tag="vcols")
        for h in range(2):
            vps = ps_tr.tile([P, 8], fp32, tag="pst")
            nc.tensor.transpose(vps, vrows[:, h * P:(h + 1) * P], ident[0:8, 0:8])
            nc.vector.tensor_copy(out=vcols[:, h, :], in_=vps)
        b1a_sb = vcols[:, :, 0]
        b1b_sb = vcols[:, :, 1]
        g1_sb = vcols[:, :, 2]
        be1_sb = vcols[:, :, 3]
        b2a_sb = vcols[:, :, 4]
        b2b_sb = vcols[:, 0, 5:6]
        g2_sb = vcols[:, 0, 6:7]
        be2_sb = vcols[:, 0, 7:8]

        # c1 = W1a.T @ S1 ; fused bias for columns >= 1
        b1ac1 = main.tile([P, 2], fp32, tag="b1ac1")
        for m in range(2):
            psc = ps_mm.tile([P, 1], fp32, tag="mm")
            nc.tensor.matmul(psc, W1a_st[:, m * P:(m + 1) * P], S1, start=True, stop=True)
            nc.vector.tensor_add(out=b1ac1[:, m:m + 1], in0=b1a_sb[:, m:m + 1], in1=psc)

        # relu1A: m=0 on ACT, m=1 on DVE (parallel)
        a1T = main.tile([P, 2, N], f32r, tag="a1T")
        nc.scalar.activation(a1T[:, 0, :], ps1a[0], RELU, bias=b1ac1[:, 0:1])
        nc.scalar.activation(a1T[:, 0, 0:1], ps1a[0][:, 0:1], RELU, bias=b1a_sb[:, 0:1])
        nc.vector.tensor_scalar(out=a1T[:, 1, :], in0=ps1a[1],
                                scalar1=b1ac1[:, 1:2], scalar2=0.0, op0=ADD, op1=MAX)
        nc.vector.tensor_scalar(out=a1T[:, 1, 0:1], in0=ps1a[1][:, 0:1],
                                scalar1=b1a_sb[:, 1:2], scalar2=0.0, op0=ADD, op1=MAX)

        # ---- MLP1 layer B ------------------------------------------------
        r1T = main.tile([P, 2, N], fp32, tag="r1T")
        ps1b = []
        for m in range(2):
            ps = ps_mm.tile([P, N], fp32, tag="mm")
            for c in range(2):
                for k in range(2):
                    nc.tensor.matmul(
                        ps[:, c * CH:(c + 1) * CH],
                        W1b_sb[:, k, m * P:(m + 1) * P],
                        a1T[:, k, c * CH:(c + 1) * CH],
                        start=(k == 0), stop=(k == 1),
                    )
            ps1b.append(ps)
        # relu1B: m=0 ACT, m=1 DVE
        nc.scalar.activation(r1T[:, 0, :], ps1b[0], RELU, bias=b1b_sb[:, 0:1])
        nc.vector.tensor_scalar(out=r1T[:, 1, :], in0=ps1b[1],
                                scalar1=b1b_sb[:, 1:2], scalar2=0.0, op0=ADD, op1=MAX)

        # c2 = W2a.T @ be1 (second aggregation: S2 = N * be1 exactly) -----
        b2ac2 = main.tile([P, 2], fp32, tag="b2ac2")
        for m in range(2):
            psc = ps_mm.tile([P, 1], fp32, tag="mm")
            for k in range(2):
                nc.tensor.matmul(
                    psc, W2a_st[:, k, m * P:(m + 1) * P], be1_sb[:, k:k + 1],
                    start=(k == 0), stop=(k == 1),
                )
            nc.vector.tensor_scalar(
                out=b2ac2[:, m:m + 1], in0=psc,
                scalar1=float(N), scalar2=b2a_sb[:, m:m + 1], op0=MULT, op1=ADD,
            )

        # ---- BN1 ---------------------------------------------------------
        st1 = main.tile([P, 2, 2, 6], fp32, tag="st1")
        mv1 = main.tile([P, 2, 2], fp32, tag="mv1")
        sg1 = main.tile([P, 2], fp32, tag="sg1")
        t1 = main.tile([P, 2], fp32, tag="t1")
        tmp1 = main.tile([P, 2], fp32, tag="tmp1")
        for m in range(2):
            for sgp in range(2):
                nc.vector.bn_stats(st1[:, m, sgp, :], r1T[:, m, sgp * CH:(sgp + 1) * CH])
            nc.vector.bn_aggr(mv1[:, m, :], st1[:, m, :, :])
            nc.scalar.activation(tmp1[:, m:m + 1], mv1[:, m, 1:2], SQRT, bias=eps_sb)
            nc.vector.reciprocal(tmp1[:, m:m + 1], tmp1[:, m:m + 1])
            nc.vector.tensor_mul(sg1[:, m:m + 1], tmp1[:, m:m + 1], g1_sb[:, m:m + 1])
            nc.vector.tensor_mul(tmp1[:, m:m + 1], mv1[:, m, 0:1], sg1[:, m:m + 1])
            nc.vector.tensor_sub(t1[:, m:m + 1], be1_sb[:, m:m + 1], tmp1[:, m:m + 1])

        bn1T = main.tile([P, 2, N], f32r, tag="bn1T")
        nc.gpsimd.tensor_scalar(out=bn1T[:, 0, :], in0=r1T[:, 0, :],
                                scalar1=sg1[:, 0:1], scalar2=t1[:, 0:1], op0=MULT, op1=ADD)
        nc.vector.tensor_scalar(out=bn1T[:, 1, :], in0=r1T[:, 1, :],
                                scalar1=sg1[:, 1:2], scalar2=t1[:, 1:2], op0=MULT, op1=ADD)

        # ---- MLP2 layer A ------------------------------------------------
        ps2a = []
        for m in range(2):
            ps = ps_mm.tile([P, N], fp32, tag="mm")
            for c in range(2):
                for k in range(2):
                    nc.tensor.matmul(
                        ps[:, c * CH:(c + 1) * CH],
                        W2a_sb[:, k, m * P:(m + 1) * P],
                        bn1T[:, k, c * CH:(c + 1) * CH],
                        start=(k == 0), stop=(k == 1),
                    )
            ps2a.append(ps)

        # relu2A: m=0 ACT, m=1 DVE
        a2T = main.tile([P, 2, N], f32r, tag="a2T")
        nc.scalar.activation(a2T[:, 0, :], ps2a[0], RELU, bias=b2ac2[:, 0:1])
        nc.scalar.activation(a2T[:, 0, 0:1], ps2a[0][:, 0:1], RELU, bias=b2a_sb[:, 0:1])
        nc.vector.tensor_scalar(out=a2T[:, 1, :], in0=ps2a[1],
                                scalar1=b2ac2[:, 1:2], scalar2=0.0, op0=ADD, op1=MAX)
        nc.vector.tensor_scalar(out=a2T[:, 1, 0:1], in0=ps2a[1][:, 0:1],
                                scalar1=b2a_sb[:, 1:2], scalar2=0.0, op0=ADD, op1=MAX)

        # ---- MLP2 layer B ------------------------------------------------
        ps2b = ps_mm.tile([P, N], fp32, tag="mm")
        for c in range(2):
            for k in range(2):
                nc.tensor.matmul(
                    ps2b[:, c * CH:(c + 1) * CH],
                    W2b_sb[:, k, :],
                    a2T[:, k, c * CH:(c + 1) * CH],
                    start=(k == 0), stop=(k == 1),
                )
        r2T = main.tile([P, N], fp32, tag="r2T")
        nc.scalar.activation(r2T, ps2b, RELU, bias=b2b_sb)

        # ---- BN2 + readout sum ------------------------------------------
        st2 = main.tile([P, 2, 6], fp32, tag="st2")
        mv2 = main.tile([P, 2], fp32, tag="mv2")
        sg2 = main.tile([P, 1], fp32, tag="sg2")
        t2 = main.tile([P, 1], fp32, tag="t2")
        tmp2 = main.tile([P, 1], fp32, tag="tmp2")
        for sgp in range(2):
            nc.vector.bn_stats(st2[:, sgp, :], r2T[:, sgp * CH:(sgp + 1) * CH])
        nc.vector.bn_aggr(mv2, st2)
        nc.scalar.activation(tmp2, mv2[:, 1:2], SQRT, bias=eps_sb)
        nc.vector.reciprocal(tmp2, tmp2)
        nc.vector.tensor_mul(sg2, tmp2, g2_sb)
        nc.vector.tensor_mul(tmp2, mv2[:, 0:1], sg2)
        nc.vector.tensor_sub(t2, be2_sb, tmp2)

        # sum over nodes of (r2*s + t) == s*(N*mean2) + N*t  (distributive)
        outv = main.tile([P, 1], fp32, tag="outv")
        suma = main.tile([P, 1], fp32, tag="suma")
        nc.vector.tensor_scalar(out=suma, in0=mv2[:, 0:1], scalar1=float(N),
                                scalar2=sg2, op0=MULT, op1=MULT)
        nc.vector.tensor_scalar(out=outv, in0=t2, scalar1=float(N),
                                scalar2=suma, op0=MULT, op1=ADD)

        nc.sync.dma_start(out=out_h[:].rearrange("(p o) -> p o", o=1), in_=outv)

        if debug:
            nc.sync.dma_start(out=dbg["S1"][:].rearrange("(p o) -> p o", o=1), in_=S1)
            nc.sync.dma_start(out=dbg["mv1"][:, :], in_=mv1[:, :, :].rearrange("p a b -> p (a b)"))
            nc.sync.dma_start(out=dbg["bn1T"][:, :, :], in_=bn1T[:, :, :].bitcast(fp32))
            nc.sync.dma_start(out=dbg["r2T"][:, :], in_=r2T[:, :])

    nc.finalize()
    return nc


_IN_NAMES = ["x", "W1a", "b1a", "W1b", "b1b", "g1", "be1",
             "W2a", "b2a", "W2b", "b2b", "g2", "be2"]

_CACHED = {}


def _get_nc(debug=False):
    key = bool(debug)
    if key not in _CACHED:
        _CACHED[key] = _build_bass(debug=debug)
    return _CACHED[key]


def kernel(**inputs) -> np.ndarray:
    from concourse.bass_utils import run_bass_kernel_spmd

    nc = _get_nc(debug=False)
    in_map = {k: np.ascontiguousarray(np.asarray(inputs[k], dtype=np.float32))
              for k in _IN_NAMES}
    in_maps = [in_map for _ in range(NCORES)]
    res = run_bass_kernel_spmd(nc, in_maps, core_ids=list(range(NCORES)))
    return np.asarray(res.results[0]["out"], dtype=np.float32)


if __name__ == "__main__":
    nc = _build_bass(debug=False)
    print("build ok:", nc)


# revision 17
# speedup vs baseline: 1.0088x; 1.0088x over previous
# Trainium2 Bass kernel for nn_MergeEncoder_47768626266491 (GIN message passing).
#
# Math notes that shape the implementation:
#   * The edge list is dense: src = repeat(arange(N), N-1), dst = tile(arange(1,N), N).
#     Every destination node d >= 1 receives ALL N source nodes, node 0 receives none.
#     So segment_sum(h[src], dst) == broadcast(colsum(h)) for rows 1..N-1, 0 for row 0.
#   * By linearity, (x + 1_{i>0} S) @ W = x@W + 1_{i>0} (S@W): the aggregation is folded
#     into a per-output-channel bias correction c = W.T @ S applied to columns >= 1.
#   * colsum(bn1_out) == N * be1 exactly (BN output sums to zero per channel), so the
#     second aggregation's S2 = N*be1 without a reduction over bn1.
#
# Distribution: the problem is tiny (1.3 MB of I/O, ~0.4 GFLOP total); the 8-core
# all-reduce latency floor (~10 us per collective, 3 would be needed) far exceeds the
# whole computation. Each core therefore runs the full network redundantly (SPMD,
# no collectives) and the host takes core 0's output.
#
# Layout: channels on partitions, nodes on the free axis (x is transposed on the PE).
# Matmuls run in float32r (1 cycle/row on the PE vs 4 for fp32); f32r is a rounded
# 4-byte format, so every tile feeding an f32r matmul is *written* as f32r by its
# producer (walrus verifies this).

import numpy as np

N, F, H = 1024, 128, 256
NCORES = 8
BN_EPS = 1e-5
P = 128  # partitions


def _build_bass(debug=False):
    from contextlib import ExitStack

    import concourse.bacc as bacc
    import concourse.tile as tile
    from concourse import mybir
    from concourse.masks import make_identity
    import concourse.hw_specs as hw_specs

    fp32 = mybir.dt.float32
    f32r = mybir.dt.float32r
    RELU = mybir.ActivationFunctionType.Relu
    SQRT = mybir.ActivationFunctionType.Sqrt
    ADD = mybir.AluOpType.add
    MULT = mybir.AluOpType.mult
    MAX = mybir.AluOpType.max
    X = mybir.AxisListType.X

    # All ACT functions used here (Relu/Sqrt/Copy) live in the sqrt_and_others
    # set; the default chooser assigns Relu to set 0 which costs a second 1.3us
    # ACT_TABLE_LOAD mid-kernel. Blank every other set (list positions, and
    # hence act_func_set_ids, are preserved) so one load covers everything.
    _orig_tables = hw_specs.get_activation_tables("gen3")

    def _only_sqrt_tables(arch):
        return {name: (fns if name == "sqrt_and_others" else set())
                for name, fns in _orig_tables.items()}

    bacc.get_activation_tables = _only_sqrt_tables

    nc = bacc.Bacc()

    x_h = nc.declare_dram_parameter("x", [N, F], fp32, isOutput=False)
    W1a_h = nc.declare_dram_parameter("W1a", [F, H], fp32, isOutput=False)
    b1a_h = nc.declare_dram_parameter("b1a", [H], fp32, isOutput=False)
    W1b_h = nc.declare_dram_parameter("W1b", [H, H], fp32, isOutput=False)
    b1b_h = nc.declare_dram_parameter("b1b", [H], fp32, isOutput=False)
    g1_h = nc.declare_dram_parameter("g1", [H], fp32, isOutput=False)
    be1_h = nc.declare_dram_parameter("be1", [H], fp32, isOutput=False)
    W2a_h = nc.declare_dram_parameter("W2a", [H, H], fp32, isOutput=False)
    b2a_h = nc.declare_dram_parameter("b2a", [H], fp32, isOutput=False)
    W2b_h = nc.declare_dram_parameter("W2b", [H, F], fp32, isOutput=False)
    b2b_h = nc.declare_dram_parameter("b2b", [F], fp32, isOutput=False)
    g2_h = nc.declare_dram_parameter("g2", [F], fp32, isOutput=False)
    be2_h = nc.declare_dram_parameter("be2", [F], fp32, isOutput=False)
    out_h = nc.declare_dram_parameter("out", [F], fp32, isOutput=True)

    dbg = {}
    if debug:
        dbg["S1"] = nc.declare_dram_parameter("dbg_S1", [P], fp32, isOutput=True)
        dbg["mv1"] = nc.declare_dram_parameter("dbg_mv1", [P, 4], fp32, isOutput=True)
        dbg["bn1T"] = nc.declare_dram_parameter("dbg_bn1T", [P, 2, N], fp32, isOutput=True)
        dbg["r2T"] = nc.declare_dram_parameter("dbg_r2T", [P, N], fp32, isOutput=True)

    NT = N // P  # 8 node tiles
    CH = 512  # node-chunk width for matmuls

    with tile.TileContext(nc) as tc, ExitStack() as ctx:
        main = ctx.enter_context(tc.tile_pool(name="main", bufs=1))
        ps_tr = ctx.enter_context(tc.tile_pool(name="ps_tr", bufs=2, space="PSUM"))
        ps_mm = ctx.enter_context(tc.tile_pool(name="ps_mm", bufs=3, space="PSUM"))

        # ---- loads ------------------------------------------------------
        ident = main.tile([P, P], fp32, tag="ident")
        make_identity(nc, ident)

        # x: 8 node-tile DMAs alternating across the two HWDGE rings so the
        # first transpose can start as soon as tile 0 lands.
        xt = main.tile([P, NT, P], fp32, tag="xt")
        for t in range(NT):
            eng = nc.sync if t % 2 == 0 else nc.scalar
            eng.dma_start(out=xt[:, t, :], in_=x_h[t * P:(t + 1) * P, :])

        W1a_st = main.tile([P, H], fp32, tag="W1a_st")
        nc.sync.dma_start(out=W1a_st, in_=W1a_h[:, :])
        W1b_st = main.tile([P, 2, H], fp32, tag="W1b_st")
        nc.sync.dma_start(out=W1b_st, in_=W1b_h[:, :].rearrange("(k p) h -> p k h", p=P))
        W2a_st = main.tile([P, 2, H], fp32, tag="W2a_st")
        nc.sync.dma_start(out=W2a_st, in_=W2a_h[:, :].rearrange("(k p) h -> p k h", p=P))
        W2b_st = main.tile([P, 2, F], fp32, tag="W2b_st")
        nc.sync.dma_start(out=W2b_st, in_=W2b_h[:, :].rearrange("(k p) h -> p k h", p=P))

        # bias/scale vectors: one single-row DMA each on the gpsimd SWDGE ring,
        # transposed to per-partition columns on the PE later.
        vrows = main.tile([8, H], fp32, tag="vrows")
        vhandles = [b1a_h, b1b_h, g1_h, be1_h, b2a_h, b2b_h, g2_h, be2_h]
        for k, h_ in enumerate(vhandles):
            L = h_.shape[0]
            nc.gpsimd.dma_start(out=vrows[k:k + 1, 0:L],
                                in_=h_[:].rearrange("(o f) -> o f", o=1))

        eps_sb = main.tile([P, 1], fp32, tag="eps")
        nc.vector.memset(eps_sb, BN_EPS)
        # dummy sqrt: forces the single ACT table load to happen up front,
        # overlapped with the input DMAs
        warm_sb = main.tile([P, 1], fp32, tag="warm")
        nc.scalar.activation(warm_sb, eps_sb, SQRT)

        # PE warm-up spin: ~4us of dummy transposes on a scratch tile while the
        # input DMAs stream in. The HAM clock gate needs ~3.4us of sustained PE
        # activity to lift the PE from 1.2 to 2.4 GHz; without this every
        # matmul in the kernel runs at half rate.
        spin_sb = main.tile([P, P], fp32, tag="spin")
        nc.vector.memset(spin_sb, 1.0)
        _warm_tr = None
        wps = ps_tr.tile([P, P], fp32, tag="pst")
        for w in range(9):
            # real matmuls: transpose-mode does not engage the HAM clock gate
            _warm_tr = nc.tensor.matmul(wps, spin_sb, spin_sb, start=True, stop=True)

        # f32r copies of the weights (DVE cast, 2x mode)
        W1a_sb = main.tile([P, H], f32r, tag="W1a")
        nc.vector.tensor_copy(out=W1a_sb, in_=W1a_st)
        W1b_sb = main.tile([P, 2, H], f32r, tag="W1b")
        nc.vector.tensor_copy(out=W1b_sb, in_=W1b_st)
        W2a_sb = main.tile([P, 2, H], f32r, tag="W2a")
        nc.gpsimd.tensor_copy(out=W2a_sb, in_=W2a_st)
        W2b_sb = main.tile([P, 2, F], f32r, tag="W2b")
        nc.gpsimd.tensor_copy(out=W2b_sb, in_=W2b_st)

        # ---- transpose x -> xT [128 ch, 1024 nodes] (f32r) --------------
        # evacuations split: even tiles on DVE, odd tiles on ACT
        xT = main.tile([P, N], f32r, tag="xT")
        S1p = main.tile([P, NT], fp32, tag="S1p")
        S1 = main.tile([P, 1], fp32, tag="S1")
        CPY = mybir.ActivationFunctionType.Copy
        _last_tr = None
        for t in range(NT):
            pst = ps_tr.tile([P, P], fp32, tag="pst")
            _last_tr = nc.tensor.transpose(pst, xt[:, t, :], ident)
            if t == 0 and _warm_tr is not None:
                from concourse.tile_rust import add_dep_helper as _adh
                _adh(_warm_tr.ins, _last_tr.ins, sync=False,
                     reason="warm-up spins before real transposes")
            # evac on ACT with a free running row-sum -> S1 partials
            nc.scalar.activation(xT[:, t * P:(t + 1) * P].bitcast(f32r), pst, CPY,
                                 accum_out=S1p[:, t:t + 1])
        nc.vector.reduce_sum(out=S1, in_=S1p, axis=X)

        # ---- MLP1 layer A ------------------------------------------------
        from concourse.tile_rust import add_dep_helper
        ps1a = []
        for m in range(2):
            ps = ps_mm.tile([P, N], fp32, tag="mm")
            for c in range(2):
                mm = nc.tensor.matmul(
                    ps[:, c * CH:(c + 1) * CH],
                    W1a_sb[:, m * P:(m + 1) * P],
                    xT[:, c * CH:(c + 1) * CH],
                    start=True, stop=True,
                )
                if m == 0 and c == 0 and _last_tr is not None:
                    # keep all transposes ahead of the MLP in the PE stream so
                    # S1 (and with it the c1 bias path) resolves early
                    add_dep_helper(_last_tr.ins, mm.ins, sync=False,
                                   reason="transposes before MLP1A")
            ps1a.append(ps)

        # vector transposes (after MM1A in the PE stream; only needed for the
        # relu biases)
        vcols = main.tile([P, 2, 8], fp32, tag="vcols")
        for h in range(2):
            vps = ps_tr.tile([P, 8], fp32, tag="pst")
            nc.tensor.transpose(vps, vrows[:, h * P:(h + 1) * P], ident[0:8, 0:8])
            nc.vector.tensor_copy(out=vcols[:, h, :], in_=vps)
        b1a_sb = vcols[:, :, 0]
        b1b_sb = vcols[:, :, 1]
        g1_sb = vcols[:, :, 2]
        be1_sb = vcols[:, :, 3]
        b2a_sb = vcols[:, :, 4]
        b2b_sb = vcols[:, 0, 5:6]
        g2_sb = vcols[:, 0, 6:7]
        be2_sb = vcols[:, 0, 7:8]

        # c1 = W1a.T @ S1 ; fused bias for columns >= 1
        b1ac1 = main.tile([P, 2], fp32, tag="b1ac1")
        for m in range(2):
            psc = ps_mm.tile([P, 1], fp32, tag="mm")
            nc.tensor.matmul(psc, W1a_st[:, m * P:(m + 1) * P], S1, start=True, stop=True)
            nc.vector.tensor_add(out=b1ac1[:, m:m + 1], in0=b1a_sb[:, m:m + 1], in1=psc)

        # relu1A: m=0 on ACT, m=1 on DVE (parallel)
        a1T = main.tile([P, 2, N], f32r, tag="a1T")
        nc.scalar.activation(a1T[:, 0, :], ps1a[0], RELU, bias=b1ac1[:, 0:1])
        nc.scalar.activation(a1T[:, 0, 0:1], ps1a[0][:, 0:1], RELU, bias=b1a_sb[:, 0:1])
        nc.vector.tensor_scalar(out=a1T[:, 1, :], in0=ps1a[1],
                                scalar1=b1ac1[:, 1:2], scalar2=0.0, op0=ADD, op1=MAX)
        nc.vector.tensor_scalar(out=a1T[:, 1, 0:1], in0=ps1a[1][:, 0:1],
                                scalar1=b1a_sb[:, 1:2], scalar2=0.0, op0=ADD, op1=MAX)

        # ---- MLP1 layer B ------------------------------------------------
        r1T = main.tile([P, 2, N], fp32, tag="r1T")
        ps1b = []
        for m in range(2):
            ps = ps_mm.tile([P, N], fp32, tag="mm")
            for c in range(2):
                for k in range(2):
                    nc.tensor.matmul(
                        ps[:, c * CH:(c + 1) * CH],
                        W1b_sb[:, k, m * P:(m + 1) * P],
                        a1T[:, k, c * CH:(c + 1) * CH],
                        start=(k == 0), stop=(k == 1),
                    )
            ps1b.append(ps)
        # relu1B: m=0 ACT, m=1 DVE
        nc.scalar.activation(r1T[:, 0, :], ps1b[0], RELU, bias=b1b_sb[:, 0:1])
        nc.vector.tensor_scalar(out=r1T[:, 1, :], in0=ps1b[1],
                                scalar1=b1b_sb[:, 1:2], scalar2=0.0, op0=ADD, op1=MAX)

        # c2 = W2a.T @ be1 (second aggregation: S2 = N * be1 exactly) -----
        b2ac2 = main.tile([P, 2], fp32, tag="b2ac2")
        for m in range(2):
            psc = ps_mm.tile([P, 1], fp32, tag="mm")
            for k in range(2):
                nc.tensor.matmul(
                    psc, W2a_st[:, k, m * P:(m + 1) * P], be1_sb[:, k:k + 1],
                    start=(k == 0), stop=(k == 1),
                )
            nc.vector.tensor_scalar(
                out=b2ac2[:, m:m + 1], in0=psc,
                scalar1=float(N), scalar2=b2a_sb[:, m:m + 1], op0=MULT, op1=ADD,
            )

        # ---- BN1 ---------------------------------------------------------
        st1 = main.tile([P, 2, 2, 6], fp32, tag="st1")
        mv1 = main.tile([P, 2, 2], fp32, tag="mv1")
        sg1 = main.tile([P, 2], fp32, tag="sg1")
        t1 = main.tile([P, 2], fp32, tag="t1")
        tmp1 = main.tile([P, 2], fp32, tag="tmp1")
        for m in range(2):
            for sgp in range(2):
                nc.vector.bn_stats(st1[:, m, sgp, :], r1T[:, m, sgp * CH:(sgp + 1) * CH])
            nc.vector.bn_aggr(mv1[:, m, :], st1[:, m, :, :])
            nc.scalar.activation(tmp1[:, m:m + 1], mv1[:, m, 1:2], SQRT, bias=eps_sb)
            nc.vector.reciprocal(tmp1[:, m:m + 1], tmp1[:, m:m + 1])
            nc.vector.tensor_mul(sg1[:, m:m + 1], tmp1[:, m:m + 1], g1_sb[:, m:m + 1])
            nc.vector.tensor_mul(tmp1[:, m:m + 1], mv1[:, m, 0:1], sg1[:, m:m + 1])
            nc.vector.tensor_sub(t1[:, m:m + 1], be1_sb[:, m:m + 1], tmp1[:, m:m + 1])

        bn1T = main.tile([P, 2, N], f32r, tag="bn1T")
        nc.gpsimd.tensor_scalar(out=bn1T[:, 0, :], in0=r1T[:, 0, :],
                                scalar1=sg1[:, 0:1], scalar2=t1[:, 0:1], op0=MULT, op1=ADD)
        nc.vector.tensor_scalar(out=bn1T[:, 1, :], in0=r1T[:, 1, :],
                                scalar1=sg1[:, 1:2], scalar2=t1[:, 1:2], op0=MULT, op1=ADD)

        # ---- MLP2 layer A ------------------------------------------------
        ps2a = []
        for m in range(2):
            ps = ps_mm.tile([P, N], fp32, tag="mm")
            for c in range(2):
                for k in range(2):
                    nc.tensor.matmul(
                        ps[:, c * CH:(c + 1) * CH],
                        W2a_sb[:, k, m * P:(m + 1) * P],
                        bn1T[:, k, c * CH:(c + 1) * CH],
                        start=(k == 0), stop=(k == 1),
                    )
            ps2a.append(ps)

        # relu2A: m=0 ACT, m=1 DVE
        a2T = main.tile([P, 2, N], f32r, tag="a2T")
        nc.scalar.activation(a2T[:, 0, :], ps2a[0], RELU, bias=b2ac2[:, 0:1])
        nc.scalar.activation(a2T[:, 0, 0:1], ps2a[0][:, 0:1], RELU, bias=b2a_sb[:, 0:1])
        nc.vector.tensor_scalar(out=a2T[:, 1, :], in0=ps2a[1],
                                scalar1=b2ac2[:, 1:2], scalar2=0.0, op0=ADD, op1=MAX)
        nc.vector.tensor_scalar(out=a2T[:, 1, 0:1], in0=ps2a[1][:, 0:1],
                                scalar1=b2a_sb[:, 1:2], scalar2=0.0, op0=ADD, op1=MAX)

        # ---- MLP2 layer B ------------------------------------------------
        ps2b = ps_mm.tile([P, N], fp32, tag="mm")
        for c in range(2):
            for k in range(2):
                nc.tensor.matmul(
                    ps2b[:, c * CH:(c + 1) * CH],
                    W2b_sb[:, k, :],
                    a2T[:, k, c * CH:(c + 1) * CH],
                    start=(k == 0), stop=(k == 1),
                )
        r2T = main.tile([P, N], fp32, tag="r2T")
        nc.scalar.activation(r2T, ps2b, RELU, bias=b2b_sb)

        # ---- BN2 + readout sum ------------------------------------------
        st2 = main.tile([P, 2, 6], fp32, tag="st2")
        mv2 = main.tile([P, 2], fp32, tag="mv2")
        sg2 = main.tile([P, 1], fp32, tag="sg2")
        t2 = main.tile([P, 1], fp32, tag="t2")
        tmp2 = main.tile([P, 1], fp32, tag="tmp2")
        for sgp in range(2):
            nc.vector.bn_stats(st2[:, sgp, :], r2T[:, sgp * CH:(sgp + 1) * CH])
        nc.vector.bn_aggr(mv2, st2)
        nc.scalar.activation(tmp2, mv2[:, 1:2], SQRT, bias=eps_sb)
        nc.vector.reciprocal(tmp2, tmp2)
        nc.vector.tensor_mul(sg2, tmp2, g2_sb)
        nc.vector.tensor_mul(tmp2, mv2[:, 0:1], sg2)
        nc.vector.tensor_sub(t2, be2_sb, tmp2)

        # sum over nodes of (r2*s + t) == s*(N*mean2) + N*t  (distributive)
        outv = main.tile([P, 1], fp32, tag="outv")
        suma = main.tile([P, 1], fp32, tag="suma")
        nc.vector.tensor_scalar(out=suma, in0=mv2[:, 0:1], scalar1=float(N),
                                scalar2=sg2, op0=MULT, op1=MULT)
        nc.vector.tensor_scalar(out=outv, in0=t2, scalar1=float(N),
                                scalar2=suma, op0=MULT, op1=ADD)

        nc.sync.dma_start(out=out_h[:].rearrange("(p o) -> p o", o=1), in_=outv)

        if debug:
            nc.sync.dma_start(out=dbg["S1"][:].rearrange("(p o) -> p o", o=1), in_=S1)
            nc.sync.dma_start(out=dbg["mv1"][:, :], in_=mv1[:, :, :].rearrange("p a b -> p (a b)"))
            nc.sync.dma_start(out=dbg["bn1T"][:, :, :], in_=bn1T[:, :, :].bitcast(fp32))
            nc.sync.dma_start(out=dbg["r2T"][:, :], in_=r2T[:, :])

    nc.finalize()
    return nc


_IN_NAMES = ["x", "W1a", "b1a", "W1b", "b1b", "g1", "be1",
             "W2a", "b2a", "W2b", "b2b", "g2", "be2"]

_CACHED = {}


def _get_nc(debug=False):
    key = bool(debug)
    if key not in _CACHED:
        _CACHED[key] = _build_bass(debug=debug)
    return _CACHED[key]


def kernel(**inputs) -> np.ndarray:
    from concourse.bass_utils import run_bass_kernel_spmd

    nc = _get_nc(debug=False)
    in_map = {k: np.ascontiguousarray(np.asarray(inputs[k], dtype=np.float32))
              for k in _IN_NAMES}
    in_maps = [in_map for _ in range(NCORES)]
    res = run_bass_kernel_spmd(nc, in_maps, core_ids=list(range(NCORES)))
    return np.asarray(res.results[0]["out"], dtype=np.float32)


if __name__ == "__main__":
    nc = _build_bass(debug=False)
    print("build ok:", nc)
